# revision 9
# baseline (speedup 1.0000x reference)
"""Chamfer distance loss kernel for Trainium2 (Bass/Tile), 8-core SPMD.

Problem: B=8 batches of N=8192 source / M=8192 target 3-D points.
  dist[n,m] = |s_n|^2 + |t_m|^2 - 2 s.t
  chamfer[b] = mean_n min_m dist + mean_m min_n dist

Sharding: data-parallel over batch; core b handles batch b end-to-end and
emits one scalar. No cross-core communication.

Per-core pipeline:
  PE  : K=4 augmented matmul  [s,1] . [-2t, |t|^2]  -> PSUM fp32 (dist - |s|^2)
  ACT : PSUM -> SBUF fp16 cast fused with per-partition +|s|^2 bias
  DVE : fp16 2x tensor_tensor min-accumulate along both axes;
        tensor_tensor_reduce fuses the final free-dim row reduce
  PE  : transpose col accumulator for the cross-partition min; ones-matmul
        for the final partition sum
"""

import numpy as np

import concourse.bacc as bacc
import concourse.bass as bass
import concourse.mybir as mybir
import concourse.tile as tile
from concourse.bass_utils import run_bass_kernel_spmd

B = 8
N = 8192  # source points per batch
M = 8192  # target points per batch
D = 3

NT = N // 128  # 64 source tiles of 128
QCH = 2048     # ACT/DVE chunk width (4 PSUM banks)
NH = M // QCH  # 4 chunks per source tile row
BIG = 60000.0  # > any squared distance here, fp16-safe

F32 = mybir.dt.float32
F16 = mybir.dt.float16
MIN = mybir.AluOpType.min
ADD = mybir.AluOpType.add


def _build_kernel(nc: bass.Bass, src_d, tgt_d, out_d, reps=1):
    tc_ctx = tile.TileContext(nc)
    with tc_ctx as tc, tc.tile_pool(name="const", bufs=1) as cpool:
        with tc.tile_pool(name="prep", bufs=1) as prep:
            # Persistent SBUF tensors
            aug_s = cpool.tile([4, N], F32)       # rows: s_x, s_y, s_z, 1
            aug_t = cpool.tile([4, M], F32)       # rows: -2t_x, -2t_y, -2t_z, |t|^2
            ssq = cpool.tile([128, NT], F32)      # |s|^2, n = 128*c + p -> [p, c]
            col_acc = cpool.tile([128, M], F16)   # min over n of dist, [p, m]
            rowmins = cpool.tile([128, NT], F32)  # min over m of dist, [p, c]
            colmins = cpool.tile([128, NT], F32)  # per-128-m-chunk col mins
            ident = cpool.tile([128, 128], F16)   # identity for PE transpose
            ones128 = cpool.tile([128, 1], F32)   # final partition-sum weights

            id_dram = nc.inline_tensor(np.eye(128, dtype=np.float16), name="ident")
            nc.sync.dma_start(ident[:], id_dram.ap())
            nc.gpsimd.memset(ones128[:], 1.0)
            ones_dram = nc.inline_tensor(np.ones((1, N), dtype=np.float32), name="ones_row")

            # ---- input prep ----
            # coord rows via strided DMA [d, n]
            nc.sync.dma_start(aug_s[0:3, :], src_d.ap().rearrange("n d -> d n"))
            nc.sync.dma_start(aug_t[0:3, :], tgt_d.ap().rearrange("m d -> d m"))
            nc.sync.dma_start(aug_s[3:4, :], ones_dram.ap())
            # scale target rows by -2 (in place)
            nc.vector.tensor_scalar_mul(aug_t[0:3, :], aug_t[0:3, :], -2.0)

            # |t|^2 row: square scaled rows, ones-matmul with 0.25 weights
            sq_t = prep.tile([3, M], F32)
            nc.scalar.square(sq_t[:], aug_t[0:3, :])
            w025 = prep.tile([3, 1], F32)
            nc.gpsimd.memset(w025[:], 0.25)
            tsq_tmp = prep.tile([1, M], F32)
            with tc.tile_pool(name="psum_prep", bufs=2, space=bass.MemorySpace.PSUM) as pprep:
                for quarter in range(4):
                    pt = pprep.tile([1, 2048], F32)
                    for q in range(4):
                        mq = quarter * 2048 + q * 512
                        nc.tensor.matmul(
                            pt[:, q * 512:(q + 1) * 512],
                            w025[:],
                            sq_t[:, mq:mq + 512],
                        )
                    nc.scalar.copy(tsq_tmp[:, quarter * 2048:(quarter + 1) * 2048], pt[:])
            nc.sync.dma_start(aug_t[3:4, :], tsq_tmp[:])

            # |s|^2 in [128, NT] layout (n = 128*c + p)
            s3n = prep.tile([128, NT, D], F32)
            nc.sync.dma_start(s3n[:], src_d.ap().rearrange("(c p) d -> p c d", p=128))
            sq3 = prep.tile([128, NT, D], F32)
            nc.scalar.square(sq3[:], s3n[:])
            nc.vector.tensor_reduce(ssq[:], sq3[:], axis=mybir.AxisListType.X, op=ADD)

        # ---- main loop (reps>1 only for exec-time measurement) ----
        for _rep in range(reps):
          with (
            tc.tile_pool(name="dpsum", bufs=2, space=bass.MemorySpace.PSUM) as dpsum,
            tc.tile_pool(name="d16", bufs=4) as d16p,
            tc.tile_pool(name="rowacc", bufs=2) as rowp,
          ):
            for c in range(NT):
                lhsT = aug_s[:, c * 128:(c + 1) * 128]
                row_acc = rowp.tile([128, QCH], F16)
                d16_first = None
                for h in range(NH):
                    dps = dpsum.tile([128, QCH], F32)
                    for q in range(QCH // 512):
                        mq = h * QCH + q * 512
                        nc.tensor.matmul(
                            dps[:, q * 512:(q + 1) * 512],
                            lhsT,
                            aug_t[:, mq:mq + 512],
                        )
                    d16 = d16p.tile([128, QCH], F16)
                    # dist = (psum) * 1 + |s|^2, cast to fp16
                    nc.scalar.activation(
                        d16[:], dps[:],
                        mybir.ActivationFunctionType.Identity,
                        bias=ssq[:, c:c + 1], scale=1.0,
                    )
                    # column (min over n) accumulate
                    cslice = col_acc[:, h * QCH:(h + 1) * QCH]
                    if c == 0:
                        nc.vector.tensor_copy(cslice, d16[:])
                    else:
                        nc.vector.tensor_tensor(cslice, d16[:], cslice, op=MIN)
                    # row (min over m) accumulate
                    if h == 0:
                        d16_first = d16
                    elif h == 1:
                        nc.vector.tensor_tensor(
                            row_acc[:], d16_first[:], d16[:], op=MIN
                        )
                    else:
                        nc.vector.tensor_tensor(row_acc[:], d16[:], row_acc[:], op=MIN)
                # binary fold then one short 1x reduce
                nc.vector.tensor_tensor(
                    row_acc[:, 0:QCH // 2],
                    row_acc[:, 0:QCH // 2], row_acc[:, QCH // 2:QCH], op=MIN,
                )
                nc.vector.tensor_tensor(
                    row_acc[:, 0:QCH // 4],
                    row_acc[:, 0:QCH // 4], row_acc[:, QCH // 4:QCH // 2], op=MIN,
                )
                nc.vector.tensor_reduce(
                    rowmins[:, c:c + 1], row_acc[:, 0:QCH // 4],
                    axis=mybir.AxisListType.X, op=MIN,
                )

        # ---- column partition-reduce via PE transpose ----
        with tc.tile_pool(name="tpsum", bufs=4, space=bass.MemorySpace.PSUM) as tpsum:
            for c in range(NT):
                tps = tpsum.tile([128, 128], F16)
                nc.tensor.transpose(tps[:], col_acc[:, c * 128:(c + 1) * 128], ident[:])
                nc.vector.tensor_reduce(
                    colmins[:, c:c + 1], tps[:], axis=mybir.AxisListType.X, op=MIN
                )

        # ---- final scalar ----
        with (
            tc.tile_pool(name="fin", bufs=1) as fin,
            tc.tile_pool(name="fpsum", bufs=1, space=bass.MemorySpace.PSUM) as fpsum,
        ):
            sums = fin.tile([128, 2], F32)
            nc.vector.tensor_reduce(
                sums[:, 0:1], rowmins[:], axis=mybir.AxisListType.X, op=ADD
            )
            nc.vector.tensor_reduce(
                sums[:, 1:2], colmins[:], axis=mybir.AxisListType.X, op=ADD
            )
            tot = fin.tile([128, 1], F32)
            nc.vector.tensor_tensor(tot[:], sums[:, 0:1], sums[:, 1:2], op=ADD)
            ps = fpsum.tile([1, 1], F32)
            nc.tensor.matmul(ps[:], tot[:], ones128[:])
            res = fin.tile([1, 1], F32)
            nc.scalar.mul(res[:], ps[:], 1.0 / float(N))
            nc.sync.dma_start(out_d.ap(), res[:])


_NC_CACHE = {}


def _get_nc(reps=1):
    if reps not in _NC_CACHE:
        nc = bacc.Bacc("TRN2", target_bir_lowering=False, debug=False)
        src_d = nc.dram_tensor("src", [N, D], F32, kind="ExternalInput")
        tgt_d = nc.dram_tensor("tgt", [M, D], F32, kind="ExternalInput")
        out_d = nc.dram_tensor("out", [1, 1], F32, kind="ExternalOutput")
        _build_kernel(nc, src_d, tgt_d, out_d, reps=reps)
        nc.compile()
        _NC_CACHE[reps] = nc
    return _NC_CACHE[reps]


def kernel(source_points: np.ndarray, target_points: np.ndarray) -> np.ndarray:
    src = np.ascontiguousarray(np.asarray(source_points), dtype=np.float32)
    tgt = np.ascontiguousarray(np.asarray(target_points), dtype=np.float32)
    assert src.shape == (B, N, D) and tgt.shape == (B, M, D)

    nc = _get_nc()
    in_maps = [{"src": src[b], "tgt": tgt[b]} for b in range(B)]
    res = run_bass_kernel_spmd(nc, in_maps, list(range(B)))
    return np.stack(
        [res.results[b]["out"].reshape(()) for b in range(B)]
    ).astype(np.float32)


if __name__ == "__main__":
    rng = np.random.default_rng(0)
    s = rng.standard_normal((B, N, D), dtype=np.float32)
    t = rng.standard_normal((B, M, D), dtype=np.float32)
    print(kernel(s, t))


# revision 11
# speedup vs baseline: 1.2319x; 1.2319x over previous
"""Chamfer distance loss kernel for Trainium2 (Bass/Tile), 8-core SPMD.

Problem: B=8 batches of N=8192 source / M=8192 target 3-D points.
  dist[n,m] = |s_n|^2 + |t_m|^2 - 2 s.t
  chamfer[b] = mean_n min_m dist + mean_m min_n dist

Sharding: data-parallel over batch; core b handles batch b end-to-end and
emits one scalar. No cross-core communication.

Per-core pipeline:
  PE  : K=4 augmented matmul  [s,1] . [-2t, |t|^2]  -> PSUM fp32 (dist - |s|^2)
  ACT : PSUM -> SBUF fp16 cast fused with per-partition +|s|^2 bias
  DVE : fp16 2x tensor_tensor min-accumulate along both axes;
        tensor_tensor_reduce fuses the final free-dim row reduce
  PE  : transpose col accumulator for the cross-partition min; ones-matmul
        for the final partition sum
"""

import numpy as np

import concourse.bacc as bacc
import concourse.bass as bass
import concourse.mybir as mybir
import concourse.tile as tile
from concourse.bass_utils import run_bass_kernel_spmd

B = 8
N = 8192  # source points per batch
M = 8192  # target points per batch
D = 3

NT = N // 128  # 64 source tiles of 128
QCH = 2048     # ACT/DVE chunk width (4 PSUM banks)
NH = M // QCH  # 4 chunks per source tile row
BIG = 60000.0  # > any squared distance here, fp16-safe

F32 = mybir.dt.float32
F16 = mybir.dt.float16
MIN = mybir.AluOpType.min
ADD = mybir.AluOpType.add


def _build_kernel(nc: bass.Bass, src_d, tgt_d, out_d, reps=1):
    tc_ctx = tile.TileContext(nc)
    with tc_ctx as tc, tc.tile_pool(name="const", bufs=1) as cpool:
        with tc.tile_pool(name="prep", bufs=1) as prep:
            # Persistent SBUF tensors
            aug_s = cpool.tile([5, N], F32)       # rows: s_x, s_y, s_z, 1, |s|^2
            aug_t = cpool.tile([5, M], F32)       # rows: -2t_x, -2t_y, -2t_z, |t|^2, 1
            col_acc = cpool.tile([128, M], F16)   # min over n of dist, [p, m]
            rowmins = cpool.tile([128, NT], F32)  # min over m of dist, [p, c]
            colmins = cpool.tile([128, NT], F32)  # per-128-m-chunk col mins
            ident = cpool.tile([128, 128], F16)   # identity for PE transpose
            ones128 = cpool.tile([128, 1], F32)   # final partition-sum weights

            id_dram = nc.inline_tensor(np.eye(128, dtype=np.float16), name="ident")
            nc.sync.dma_start(ident[:], id_dram.ap())
            nc.gpsimd.memset(ones128[:], 1.0)
            ones_dram = nc.inline_tensor(np.ones((1, N), dtype=np.float32), name="ones_row")

            # ---- input prep ----
            # coord rows via strided DMA [d, n]
            nc.sync.dma_start(aug_s[0:3, :], src_d.ap().rearrange("n d -> d n"))
            nc.sync.dma_start(aug_t[0:3, :], tgt_d.ap().rearrange("m d -> d m"))
            nc.sync.dma_start(aug_s[3:4, :], ones_dram.ap())
            nc.sync.dma_start(aug_t[4:5, :], ones_dram.ap())
            # scale target rows by -2 (in place)
            nc.vector.tensor_scalar_mul(aug_t[0:3, :], aug_t[0:3, :], -2.0)

            # |t|^2 row: square scaled rows, ones-matmul with 0.25 weights
            sq_t = prep.tile([3, M], F32, tag="sq")
            nc.scalar.square(sq_t[:], aug_t[0:3, :])
            w025 = prep.tile([3, 1], F32)
            nc.gpsimd.memset(w025[:], 0.25)
            tsq_tmp = prep.tile([1, M], F32, tag="tmp")
            with tc.tile_pool(name="psum_prep", bufs=2, space=bass.MemorySpace.PSUM) as pprep:
                for quarter in range(4):
                    pt = pprep.tile([1, 2048], F32)
                    for q in range(4):
                        mq = quarter * 2048 + q * 512
                        nc.tensor.matmul(
                            pt[:, q * 512:(q + 1) * 512],
                            w025[:],
                            sq_t[:, mq:mq + 512],
                        )
                    nc.scalar.copy(tsq_tmp[:, quarter * 2048:(quarter + 1) * 2048], pt[:])
            nc.sync.dma_start(aug_t[3:4, :], tsq_tmp[:])

            # |s|^2 row via square + ones-matmul (weights 1.0)
            sq_s = prep.tile([3, N], F32, tag="sq")
            nc.scalar.square(sq_s[:], aug_s[0:3, :])
            w1 = prep.tile([3, 1], F32)
            nc.gpsimd.memset(w1[:], 1.0)
            ssq_tmp = prep.tile([1, N], F32, tag="tmp")
            with tc.tile_pool(name="psum_prep2", bufs=2, space=bass.MemorySpace.PSUM) as pprep2:
                for quarter in range(4):
                    pt2 = pprep2.tile([1, 2048], F32)
                    for q in range(4):
                        nq = quarter * 2048 + q * 512
                        nc.tensor.matmul(
                            pt2[:, q * 512:(q + 1) * 512],
                            w1[:],
                            sq_s[:, nq:nq + 512],
                        )
                    nc.scalar.copy(ssq_tmp[:, quarter * 2048:(quarter + 1) * 2048], pt2[:])
            nc.sync.dma_start(aug_s[4:5, :], ssq_tmp[:])

        # ---- main loop (reps>1 only for exec-time measurement) ----
        for _rep in range(reps):
          with (
            tc.tile_pool(name="dpsum", bufs=2, space=bass.MemorySpace.PSUM) as dpsum,
            tc.tile_pool(name="d16", bufs=6) as d16p,
            tc.tile_pool(name="rowacc", bufs=2) as rowp,
          ):
            for c in range(NT):
                lhsT = aug_s[:, c * 128:(c + 1) * 128]
                row_acc = rowp.tile([128, QCH], F16)
                d16_first = None
                for h in range(NH):
                    dps = dpsum.tile([128, QCH], F32)
                    for q in range(QCH // 512):
                        mq = h * QCH + q * 512
                        nc.tensor.matmul(
                            dps[:, q * 512:(q + 1) * 512],
                            lhsT,
                            aug_t[:, mq:mq + 512],
                        )
                    d16 = d16p.tile([128, QCH], F16)
                    nc.scalar.copy(d16[:], dps[:])  # fp32 PSUM -> fp16 SBUF
                    # column (min over n) accumulate
                    cslice = col_acc[:, h * QCH:(h + 1) * QCH]
                    if c == 0:
                        nc.vector.tensor_copy(cslice, d16[:])
                    else:
                        nc.vector.tensor_tensor(cslice, d16[:], cslice, op=MIN)
                    # row (min over m) accumulate
                    if h == 0:
                        d16_first = d16
                    elif h == 1:
                        nc.vector.tensor_tensor(
                            row_acc[:], d16_first[:], d16[:], op=MIN
                        )
                    else:
                        nc.vector.tensor_tensor(row_acc[:], d16[:], row_acc[:], op=MIN)
                # binary fold then one short 1x reduce
                nc.vector.tensor_tensor(
                    row_acc[:, 0:QCH // 2],
                    row_acc[:, 0:QCH // 2], row_acc[:, QCH // 2:QCH], op=MIN,
                )
                nc.vector.tensor_tensor(
                    row_acc[:, 0:QCH // 4],
                    row_acc[:, 0:QCH // 4], row_acc[:, QCH // 4:QCH // 2], op=MIN,
                )
                nc.vector.tensor_reduce(
                    rowmins[:, c:c + 1], row_acc[:, 0:QCH // 4],
                    axis=mybir.AxisListType.X, op=MIN,
                )

        # ---- column partition-reduce via PE transpose ----
        with tc.tile_pool(name="tpsum", bufs=4, space=bass.MemorySpace.PSUM) as tpsum:
            for c in range(NT):
                tps = tpsum.tile([128, 128], F16)
                nc.tensor.transpose(tps[:], col_acc[:, c * 128:(c + 1) * 128], ident[:])
                nc.vector.tensor_reduce(
                    colmins[:, c:c + 1], tps[:], axis=mybir.AxisListType.X, op=MIN
                )

        # ---- final scalar ----
        with (
            tc.tile_pool(name="fin", bufs=1) as fin,
            tc.tile_pool(name="fpsum", bufs=1, space=bass.MemorySpace.PSUM) as fpsum,
        ):
            sums = fin.tile([128, 2], F32)
            nc.vector.tensor_reduce(
                sums[:, 0:1], rowmins[:], axis=mybir.AxisListType.X, op=ADD
            )
            nc.vector.tensor_reduce(
                sums[:, 1:2], colmins[:], axis=mybir.AxisListType.X, op=ADD
            )
            tot = fin.tile([128, 1], F32)
            nc.vector.tensor_tensor(tot[:], sums[:, 0:1], sums[:, 1:2], op=ADD)
            ps = fpsum.tile([1, 1], F32)
            nc.tensor.matmul(ps[:], tot[:], ones128[:])
            res = fin.tile([1, 1], F32)
            nc.scalar.mul(res[:], ps[:], 1.0 / float(N))
            nc.sync.dma_start(out_d.ap(), res[:])


_NC_CACHE = {}


def _get_nc(reps=1):
    if reps not in _NC_CACHE:
        nc = bacc.Bacc("TRN2", target_bir_lowering=False, debug=False)
        src_d = nc.dram_tensor("src", [N, D], F32, kind="ExternalInput")
        tgt_d = nc.dram_tensor("tgt", [M, D], F32, kind="ExternalInput")
        out_d = nc.dram_tensor("out", [1, 1], F32, kind="ExternalOutput")
        _build_kernel(nc, src_d, tgt_d, out_d, reps=reps)
        nc.compile()
        _NC_CACHE[reps] = nc
    return _NC_CACHE[reps]


def kernel(source_points: np.ndarray, target_points: np.ndarray) -> np.ndarray:
    src = np.ascontiguousarray(np.asarray(source_points), dtype=np.float32)
    tgt = np.ascontiguousarray(np.asarray(target_points), dtype=np.float32)
    assert src.shape == (B, N, D) and tgt.shape == (B, M, D)

    nc = _get_nc()
    in_maps = [{"src": src[b], "tgt": tgt[b]} for b in range(B)]
    res = run_bass_kernel_spmd(nc, in_maps, list(range(B)))
    return np.stack(
        [res.results[b]["out"].reshape(()) for b in range(B)]
    ).astype(np.float32)


if __name__ == "__main__":
    rng = np.random.default_rng(0)
    s = rng.standard_normal((B, N, D), dtype=np.float32)
    t = rng.standard_normal((B, M, D), dtype=np.float32)
    print(kernel(s, t))


# revision 12
# speedup vs baseline: 1.2398x; 1.0064x over previous
"""Chamfer distance loss kernel for Trainium2 (Bass/Tile), 8-core SPMD.

Problem: B=8 batches of N=8192 source / M=8192 target 3-D points.
  dist[n,m] = |s_n|^2 + |t_m|^2 - 2 s.t
  chamfer[b] = mean_n min_m dist + mean_m min_n dist

Sharding: data-parallel over batch; core b handles batch b end-to-end and
emits one scalar. No cross-core communication.

Per-core pipeline:
  PE  : K=4 augmented matmul  [s,1] . [-2t, |t|^2]  -> PSUM fp32 (dist - |s|^2)
  ACT : PSUM -> SBUF fp16 cast fused with per-partition +|s|^2 bias
  DVE : fp16 2x tensor_tensor min-accumulate along both axes;
        tensor_tensor_reduce fuses the final free-dim row reduce
  PE  : transpose col accumulator for the cross-partition min; ones-matmul
        for the final partition sum
"""

import numpy as np

import concourse.bacc as bacc
import concourse.bass as bass
import concourse.mybir as mybir
import concourse.tile as tile
from concourse.bass_utils import run_bass_kernel_spmd

B = 8
N = 8192  # source points per batch
M = 8192  # target points per batch
D = 3

NT = N // 128  # 64 source tiles of 128
QCH = 2048     # ACT/DVE chunk width (4 PSUM banks)
NH = M // QCH  # 4 chunks per source tile row
BIG = 60000.0  # > any squared distance here, fp16-safe

F32 = mybir.dt.float32
F16 = mybir.dt.float16
MIN = mybir.AluOpType.min
ADD = mybir.AluOpType.add


def _build_kernel(nc: bass.Bass, src_d, tgt_d, out_d, reps=1):
    tc_ctx = tile.TileContext(nc)
    with tc_ctx as tc, tc.tile_pool(name="const", bufs=1) as cpool:
        with tc.tile_pool(name="prep", bufs=1) as prep:
            # Persistent SBUF tensors
            aug_s = cpool.tile([5, N], F32)       # rows: s_x, s_y, s_z, 1, |s|^2
            aug_t = cpool.tile([5, M], F32)       # rows: -2t_x, -2t_y, -2t_z, |t|^2, 1
            col_acc = cpool.tile([128, M], F16)   # min over n of dist, [p, m]
            rowmins = cpool.tile([128, NT], F32)  # min over m of dist, [p, c]
            colmins = cpool.tile([128, NT], F32)  # per-128-m-chunk col mins
            ident = cpool.tile([128, 128], F16)   # identity for PE transpose
            ones128 = cpool.tile([128, 1], F32)   # final partition-sum weights

            id_dram = nc.inline_tensor(np.eye(128, dtype=np.float16), name="ident")
            nc.sync.dma_start(ident[:], id_dram.ap())
            nc.gpsimd.memset(ones128[:], 1.0)
            ones_dram = nc.inline_tensor(np.ones((1, N), dtype=np.float32), name="ones_row")

            # ---- input prep ----
            # coord rows via strided DMA [d, n]
            nc.sync.dma_start(aug_s[0:3, :], src_d.ap().rearrange("n d -> d n"))
            nc.sync.dma_start(aug_t[0:3, :], tgt_d.ap().rearrange("m d -> d m"))
            nc.sync.dma_start(aug_s[3:4, :], ones_dram.ap())
            nc.sync.dma_start(aug_t[4:5, :], ones_dram.ap())
            # scale target rows by -2 (in place)
            nc.vector.tensor_scalar_mul(aug_t[0:3, :], aug_t[0:3, :], -2.0)

            # |t|^2 row: square scaled rows, ones-matmul with 0.25 weights
            sq_t = prep.tile([3, M], F32, tag="sq")
            nc.scalar.square(sq_t[:], aug_t[0:3, :])
            w025 = prep.tile([3, 1], F32)
            nc.gpsimd.memset(w025[:], 0.25)
            tsq_tmp = prep.tile([1, M], F32, tag="tmp")
            with tc.tile_pool(name="psum_prep", bufs=2, space=bass.MemorySpace.PSUM) as pprep:
                for quarter in range(4):
                    pt = pprep.tile([1, 2048], F32)
                    for q in range(4):
                        mq = quarter * 2048 + q * 512
                        nc.tensor.matmul(
                            pt[:, q * 512:(q + 1) * 512],
                            w025[:],
                            sq_t[:, mq:mq + 512],
                        )
                    nc.scalar.copy(tsq_tmp[:, quarter * 2048:(quarter + 1) * 2048], pt[:])
            nc.sync.dma_start(aug_t[3:4, :], tsq_tmp[:])

            # |s|^2 row via square + ones-matmul (weights 1.0)
            sq_s = prep.tile([3, N], F32, tag="sq")
            nc.scalar.square(sq_s[:], aug_s[0:3, :])
            w1 = prep.tile([3, 1], F32)
            nc.gpsimd.memset(w1[:], 1.0)
            ssq_tmp = prep.tile([1, N], F32, tag="tmp")
            with tc.tile_pool(name="psum_prep2", bufs=2, space=bass.MemorySpace.PSUM) as pprep2:
                for quarter in range(4):
                    pt2 = pprep2.tile([1, 2048], F32)
                    for q in range(4):
                        nq = quarter * 2048 + q * 512
                        nc.tensor.matmul(
                            pt2[:, q * 512:(q + 1) * 512],
                            w1[:],
                            sq_s[:, nq:nq + 512],
                        )
                    nc.scalar.copy(ssq_tmp[:, quarter * 2048:(quarter + 1) * 2048], pt2[:])
            nc.sync.dma_start(aug_s[4:5, :], ssq_tmp[:])

        # ---- main loop (reps>1 only for exec-time measurement) ----
        for _rep in range(reps):
          with (
            tc.tile_pool(name="dpsum", bufs=2, space=bass.MemorySpace.PSUM) as dpsum,
            tc.tile_pool(name="d16", bufs=2) as d16p,
            tc.tile_pool(name="rowacc", bufs=1) as rowp,
          ):
            for c in range(NT):
                lhsT = aug_s[:, c * 128:(c + 1) * 128]
                d16 = d16p.tile([128, M], F16)
                for h in range(NH):
                    dps = dpsum.tile([128, QCH], F32)
                    for q in range(QCH // 512):
                        mq = h * QCH + q * 512
                        nc.tensor.matmul(
                            dps[:, q * 512:(q + 1) * 512],
                            lhsT,
                            aug_t[:, mq:mq + 512],
                        )
                    # fp32 PSUM -> fp16 SBUF slice of the full row block
                    nc.scalar.copy(d16[:, h * QCH:(h + 1) * QCH], dps[:])
                # column (min over n) accumulate: one wide op
                if c == 0:
                    nc.vector.tensor_copy(col_acc[:], d16[:])
                else:
                    nc.vector.tensor_tensor(col_acc[:], d16[:], col_acc[:], op=MIN)
                # row (min over m): binary fold tree then one short 1x reduce
                rowh = rowp.tile([128, M // 2], F16)
                nc.vector.tensor_tensor(
                    rowh[:], d16[:, 0:M // 2], d16[:, M // 2:M], op=MIN
                )
                for w in (M // 4, M // 8, M // 16):
                    nc.vector.tensor_tensor(
                        rowh[:, 0:w], rowh[:, 0:w], rowh[:, w:2 * w], op=MIN
                    )
                nc.vector.tensor_reduce(
                    rowmins[:, c:c + 1], rowh[:, 0:M // 16],
                    axis=mybir.AxisListType.X, op=MIN,
                )

        # ---- column partition-reduce via PE transpose ----
        with tc.tile_pool(name="tpsum", bufs=4, space=bass.MemorySpace.PSUM) as tpsum:
            for c in range(NT):
                tps = tpsum.tile([128, 128], F16)
                nc.tensor.transpose(tps[:], col_acc[:, c * 128:(c + 1) * 128], ident[:])
                nc.vector.tensor_reduce(
                    colmins[:, c:c + 1], tps[:], axis=mybir.AxisListType.X, op=MIN
                )

        # ---- final scalar ----
        with (
            tc.tile_pool(name="fin", bufs=1) as fin,
            tc.tile_pool(name="fpsum", bufs=1, space=bass.MemorySpace.PSUM) as fpsum,
        ):
            sums = fin.tile([128, 2], F32)
            nc.vector.tensor_reduce(
                sums[:, 0:1], rowmins[:], axis=mybir.AxisListType.X, op=ADD
            )
            nc.vector.tensor_reduce(
                sums[:, 1:2], colmins[:], axis=mybir.AxisListType.X, op=ADD
            )
            tot = fin.tile([128, 1], F32)
            nc.vector.tensor_tensor(tot[:], sums[:, 0:1], sums[:, 1:2], op=ADD)
            ps = fpsum.tile([1, 1], F32)
            nc.tensor.matmul(ps[:], tot[:], ones128[:])
            res = fin.tile([1, 1], F32)
            nc.scalar.mul(res[:], ps[:], 1.0 / float(N))
            nc.sync.dma_start(out_d.ap(), res[:])


_NC_CACHE = {}


def _get_nc(reps=1):
    if reps not in _NC_CACHE:
        nc = bacc.Bacc("TRN2", target_bir_lowering=False, debug=False)
        src_d = nc.dram_tensor("src", [N, D], F32, kind="ExternalInput")
        tgt_d = nc.dram_tensor("tgt", [M, D], F32, kind="ExternalInput")
        out_d = nc.dram_tensor("out", [1, 1], F32, kind="ExternalOutput")
        _build_kernel(nc, src_d, tgt_d, out_d, reps=reps)
        nc.compile()
        _NC_CACHE[reps] = nc
    return _NC_CACHE[reps]


def kernel(source_points: np.ndarray, target_points: np.ndarray) -> np.ndarray:
    src = np.ascontiguousarray(np.asarray(source_points), dtype=np.float32)
    tgt = np.ascontiguousarray(np.asarray(target_points), dtype=np.float32)
    assert src.shape == (B, N, D) and tgt.shape == (B, M, D)

    nc = _get_nc()
    in_maps = [{"src": src[b], "tgt": tgt[b]} for b in range(B)]
    res = run_bass_kernel_spmd(nc, in_maps, list(range(B)))
    return np.stack(
        [res.results[b]["out"].reshape(()) for b in range(B)]
    ).astype(np.float32)


if __name__ == "__main__":
    rng = np.random.default_rng(0)
    s = rng.standard_normal((B, N, D), dtype=np.float32)
    t = rng.standard_normal((B, M, D), dtype=np.float32)
    print(kernel(s, t))


# revision 14
# speedup vs baseline: 3.5954x; 2.9001x over previous
"""Chamfer distance loss kernel for Trainium2 (Bass/Tile), 8-core SPMD.

Problem: B=8 batches of N=8192 source / M=8192 target 3-D points.
  dist[n,m] = |s_n|^2 + |t_m|^2 - 2 s.t
  chamfer[b] = mean_n min_m dist + mean_m min_n dist

Sharding: data-parallel over batch; core b handles batch b end-to-end and
emits one scalar. No cross-core communication.

Per-core pipeline:
  PE  : K=5 augmented matmul [s,1,|s|^2].[-2t,|t|^2,1] -> PSUM fp32 = dist
  ACT : PSUM -> SBUF bf16 cast (plain Copy)
  DVE : bf16 2x tensor_tensor min-accumulate (col) + binary fold tree (row)
  PE  : transpose col accumulator for the cross-partition min; ones-matmul
        for the final partition sum
"""

import ml_dtypes
import numpy as np

import concourse.bacc as bacc
import concourse.bass as bass
import concourse.mybir as mybir
import concourse.tile as tile
from concourse.bass_utils import run_bass_kernel_spmd

B = 8
N = 8192  # source points per batch
M = 8192  # target points per batch
D = 3

NT = N // 128  # 64 source tiles of 128
QCH = 2048     # ACT/DVE chunk width (4 PSUM banks)
NH = M // QCH  # 4 chunks per source tile row
BIG = 60000.0  # > any squared distance here, fp16-safe

F32 = mybir.dt.float32
F16 = mybir.dt.bfloat16
MIN = mybir.AluOpType.min
ADD = mybir.AluOpType.add


def _build_kernel(nc: bass.Bass, src_d, tgt_d, out_d, reps=1):
    tc_ctx = tile.TileContext(nc)
    with tc_ctx as tc, tc.tile_pool(name="const", bufs=1) as cpool:
        with tc.tile_pool(name="prep", bufs=1) as prep:
            # Persistent SBUF tensors
            aug_s = cpool.tile([5, N], F32)       # rows: s_x, s_y, s_z, 1, |s|^2
            aug_t = cpool.tile([5, M], F32)       # rows: -2t_x, -2t_y, -2t_z, |t|^2, 1
            col_acc = cpool.tile([128, M], F16)   # min over n of dist, [p, m]
            rowmins = cpool.tile([128, NT], F32)  # min over m of dist, [p, c]
            colmins = cpool.tile([128, NT], F32)  # per-128-m-chunk col mins
            ident = cpool.tile([128, 128], F16)   # identity for PE transpose
            ones128 = cpool.tile([128, 1], F32)   # final partition-sum weights

            id_dram = nc.inline_tensor(np.eye(128, dtype=np.float32).astype(ml_dtypes.bfloat16), name="ident")
            nc.sync.dma_start(ident[:], id_dram.ap())
            nc.gpsimd.memset(ones128[:], 1.0)
            ones_dram = nc.inline_tensor(np.ones((1, N), dtype=np.float32), name="ones_row")

            # ---- input prep ----
            # coord rows via strided DMA [d, n]
            nc.sync.dma_start(aug_s[0:3, :], src_d.ap().rearrange("n d -> d n"))
            nc.sync.dma_start(aug_t[0:3, :], tgt_d.ap().rearrange("m d -> d m"))
            nc.sync.dma_start(aug_s[3:4, :], ones_dram.ap())
            nc.sync.dma_start(aug_t[4:5, :], ones_dram.ap())
            # scale target rows by -2 (in place)
            nc.vector.tensor_scalar_mul(aug_t[0:3, :], aug_t[0:3, :], -2.0)

            # |t|^2 row: square scaled rows, ones-matmul with 0.25 weights
            sq_t = prep.tile([3, M], F32, tag="sq")
            nc.scalar.square(sq_t[:], aug_t[0:3, :])
            w025 = prep.tile([3, 1], F32)
            nc.gpsimd.memset(w025[:], 0.25)
            tsq_tmp = prep.tile([1, M], F32, tag="tmp")
            with tc.tile_pool(name="psum_prep", bufs=2, space=bass.MemorySpace.PSUM) as pprep:
                for quarter in range(4):
                    pt = pprep.tile([1, 2048], F32)
                    for q in range(4):
                        mq = quarter * 2048 + q * 512
                        nc.tensor.matmul(
                            pt[:, q * 512:(q + 1) * 512],
                            w025[:],
                            sq_t[:, mq:mq + 512],
                        )
                    nc.scalar.copy(tsq_tmp[:, quarter * 2048:(quarter + 1) * 2048], pt[:])
            nc.sync.dma_start(aug_t[3:4, :], tsq_tmp[:])

            # |s|^2 row via square + ones-matmul (weights 1.0)
            sq_s = prep.tile([3, N], F32, tag="sq")
            nc.scalar.square(sq_s[:], aug_s[0:3, :])
            w1 = prep.tile([3, 1], F32)
            nc.gpsimd.memset(w1[:], 1.0)
            ssq_tmp = prep.tile([1, N], F32, tag="tmp")
            with tc.tile_pool(name="psum_prep2", bufs=2, space=bass.MemorySpace.PSUM) as pprep2:
                for quarter in range(4):
                    pt2 = pprep2.tile([1, 2048], F32)
                    for q in range(4):
                        nq = quarter * 2048 + q * 512
                        nc.tensor.matmul(
                            pt2[:, q * 512:(q + 1) * 512],
                            w1[:],
                            sq_s[:, nq:nq + 512],
                        )
                    nc.scalar.copy(ssq_tmp[:, quarter * 2048:(quarter + 1) * 2048], pt2[:])
            nc.sync.dma_start(aug_s[4:5, :], ssq_tmp[:])

        # ---- main loop (reps>1 only for exec-time measurement) ----
        for _rep in range(reps):
          with (
            tc.tile_pool(name="dpsum", bufs=2, space=bass.MemorySpace.PSUM) as dpsum,
            tc.tile_pool(name="d16", bufs=2) as d16p,
            tc.tile_pool(name="rowacc", bufs=1) as rowp,
          ):
            for c in range(NT):
                lhsT = aug_s[:, c * 128:(c + 1) * 128]
                d16 = d16p.tile([128, M], F16)
                for h in range(NH):
                    dps = dpsum.tile([128, QCH], F32)
                    for q in range(QCH // 512):
                        mq = h * QCH + q * 512
                        nc.tensor.matmul(
                            dps[:, q * 512:(q + 1) * 512],
                            lhsT,
                            aug_t[:, mq:mq + 512],
                        )
                    # fp32 PSUM -> fp16 SBUF slice of the full row block
                    nc.scalar.copy(d16[:, h * QCH:(h + 1) * QCH], dps[:])
                # column (min over n) accumulate: one wide op
                if c == 0:
                    nc.vector.tensor_copy(col_acc[:], d16[:])
                else:
                    nc.vector.tensor_tensor(col_acc[:], d16[:], col_acc[:], op=MIN)
                # row (min over m): binary fold tree then one short 1x reduce
                rowh = rowp.tile([128, M // 2], F16)
                nc.vector.tensor_tensor(
                    rowh[:], d16[:, 0:M // 2], d16[:, M // 2:M], op=MIN
                )
                for w in (M // 4, M // 8, M // 16):
                    nc.vector.tensor_tensor(
                        rowh[:, 0:w], rowh[:, 0:w], rowh[:, w:2 * w], op=MIN
                    )
                nc.vector.tensor_reduce(
                    rowmins[:, c:c + 1], rowh[:, 0:M // 16],
                    axis=mybir.AxisListType.X, op=MIN,
                )

        # ---- column partition-reduce via PE transpose ----
        with tc.tile_pool(name="tpsum", bufs=4, space=bass.MemorySpace.PSUM) as tpsum:
            for c in range(NT):
                tps = tpsum.tile([128, 128], F16)
                nc.tensor.transpose(tps[:], col_acc[:, c * 128:(c + 1) * 128], ident[:])
                nc.vector.tensor_reduce(
                    colmins[:, c:c + 1], tps[:], axis=mybir.AxisListType.X, op=MIN
                )

        # ---- final scalar ----
        with (
            tc.tile_pool(name="fin", bufs=1) as fin,
            tc.tile_pool(name="fpsum", bufs=1, space=bass.MemorySpace.PSUM) as fpsum,
        ):
            sums = fin.tile([128, 2], F32)
            nc.vector.tensor_reduce(
                sums[:, 0:1], rowmins[:], axis=mybir.AxisListType.X, op=ADD
            )
            nc.vector.tensor_reduce(
                sums[:, 1:2], colmins[:], axis=mybir.AxisListType.X, op=ADD
            )
            tot = fin.tile([128, 1], F32)
            nc.vector.tensor_tensor(tot[:], sums[:, 0:1], sums[:, 1:2], op=ADD)
            ps = fpsum.tile([1, 1], F32)
            nc.tensor.matmul(ps[:], tot[:], ones128[:])
            res = fin.tile([1, 1], F32)
            nc.scalar.mul(res[:], ps[:], 1.0 / float(N))
            nc.sync.dma_start(out_d.ap(), res[:])


_NC_CACHE = {}


def _get_nc(reps=1):
    if reps not in _NC_CACHE:
        nc = bacc.Bacc("TRN2", target_bir_lowering=False, debug=False)
        src_d = nc.dram_tensor("src", [N, D], F32, kind="ExternalInput")
        tgt_d = nc.dram_tensor("tgt", [M, D], F32, kind="ExternalInput")
        out_d = nc.dram_tensor("out", [1, 1], F32, kind="ExternalOutput")
        _build_kernel(nc, src_d, tgt_d, out_d, reps=reps)
        nc.compile()
        _NC_CACHE[reps] = nc
    return _NC_CACHE[reps]


def kernel(source_points: np.ndarray, target_points: np.ndarray) -> np.ndarray:
    src = np.ascontiguousarray(np.asarray(source_points), dtype=np.float32)
    tgt = np.ascontiguousarray(np.asarray(target_points), dtype=np.float32)
    assert src.shape == (B, N, D) and tgt.shape == (B, M, D)

    nc = _get_nc()
    in_maps = [{"src": src[b], "tgt": tgt[b]} for b in range(B)]
    res = run_bass_kernel_spmd(nc, in_maps, list(range(B)))
    return np.stack(
        [res.results[b]["out"].reshape(()) for b in range(B)]
    ).astype(np.float32)


if __name__ == "__main__":
    rng = np.random.default_rng(0)
    s = rng.standard_normal((B, N, D), dtype=np.float32)
    t = rng.standard_normal((B, M, D), dtype=np.float32)
    print(kernel(s, t))
